# revision 2
# baseline (speedup 1.0000x reference)
"""Causal self-attention (B=2, T=2048, C=1024, H=16) on 8 trn2 NeuronCores.

Sharding: core = (batch b, head-group g) with 4 heads per group.
  - data parallel over B (2 ways) x tensor parallel over heads (4 ways)
  - each core computes qkv for its head group, causal attention for its
    4 heads, and a partial proj (its 256 rows of w_proj); the host sums
    the 4 per-batch partials (deferred tensor-parallel all-reduce).

All SBUF operands are bf16 (PE streams 1 moving-elem/cycle at any size,
DVE gets 2x tensor_tensor, DMA bytes halve); PSUM accumulation is fp32.
Attention math is arranged so no on-device transposes are needed:
  q^T,k^T [d, t] come straight out of the qkv matmul (lhsT = w slice,
  rhs = x^T); S^T[tk,tq] = k^T.T @ q^T-moving; exp on ACT (one merged
  instruction per key tile covering both heads of a pair); y^T and the
  softmax denominators come from the AV matmul via padded 128-wide
  v-lhsT tiles:
    half0 lhsT = [v_h0 | ones | 0...]   -> y_A in psum rows 0-63, denomA row 64
    half1 lhsT = [0... | ones | v_h1]   -> denomB row 63, y_B rows 64-127
  so after the psum->sbuf copy both heads' y sit partition-aligned with
  yT and no cross-partition DMA is needed for the normalize.
Softmax skips max-subtraction (scores ~ N(0,1) after 1/sqrt(D): exp is
safe in fp32), matching the reference up to fp rounding.
"""

import os
import sys
from contextlib import ExitStack

import numpy as np

for _p in ("/opt/trn_rl_repo", "/root/.axon_site/_ro/trn_rl_repo"):
    if os.path.isdir(_p) and _p not in sys.path:
        sys.path.insert(0, _p)

import concourse.bass as bass
import concourse.bacc as bacc
import concourse.mybir as mybir
import concourse.tile as tile
from concourse.bass_utils import run_bass_kernel_spmd

B, T, C, H, D = 2, 2048, 1024, 16, 64
GH = 4                 # heads per core (group)
GC = GH * D            # 256 channels per group
NCORES = 8
TQ = 512               # query tile (free dim of S^T / AV matmuls)
TK = 128               # key tile (partition dim of S^T)
NB = T // TQ           # 4 query blocks
NT = T // TK           # 16 key tiles
CK = C // 128          # 8 contraction chunks for qkv
F32 = mybir.dt.float32
BF16 = mybir.dt.bfloat16

EXPF = mybir.ActivationFunctionType.Exp

_CACHE = {}


def _build_bass(repeat=1):
    nc = bacc.Bacc("TRN2", target_bir_lowering=False, debug=False)
    xt = nc.declare_dram_parameter("xt", [NB, CK, 128, TQ], BF16, isOutput=False)
    wqkv = nc.declare_dram_parameter("wqkv", [CK, 128, 3 * GC], BF16, isOutput=False)
    wproj = nc.declare_dram_parameter("wproj", [2, 128, C], BF16, isOutput=False)
    masks = nc.declare_dram_parameter("masks", [128, 4 * TQ], BF16, isOutput=False)
    out = nc.declare_dram_parameter("out", [NB, TQ, C], BF16, isOutput=True)

    with ExitStack() as ctx:
        tc = ctx.enter_context(tile.TileContext(nc))
        consts = ctx.enter_context(tc.tile_pool(name="consts", bufs=1))
        persist = ctx.enter_context(tc.tile_pool(name="persist", bufs=1))
        xpool = ctx.enter_context(tc.tile_pool(name="xp", bufs=2))
        espool = ctx.enter_context(tc.tile_pool(name="es", bufs=4))
        rpool = ctx.enter_context(tc.tile_pool(name="rp", bufs=2))
        rbpool = ctx.enter_context(tc.tile_pool(name="rb", bufs=2))
        opool = ctx.enter_context(tc.tile_pool(name="op", bufs=2))
        dpool = ctx.enter_context(tc.tile_pool(name="dp", bufs=2, space="DRAM"))
        # 8 fixed PSUM banks, rotated manually (pool-based PSUM slot reuse
        # emits extra release waits on the claiming matmul).
        psum = ctx.enter_context(tc.tile_pool(name="psum", bufs=1, space="PSUM"))
        PSD = [
            psum.tile([128, 2, TQ], F32, tag=f"psd{r}", name=f"psd{r}")
            for r in range(2)
        ]
        PP = [psum.tile([128, TQ], F32, tag="pp0", name="pp0")]
        PJ = [psum.tile([128, TQ], F32, tag="pj0", name="pj0")]
        AVD = psum.tile([128, 2, TQ], F32, tag="avd", name="avd")
        cnt = {"pp": 0, "pj": 0, "ps": 0}

        # ---- constants / persistent tiles ----
        w_sb = consts.tile([128, CK, 3 * GC], BF16, tag="wqkv")
        wp_sb = consts.tile([128, 2, C], BF16, tag="wproj")
        mk_sb = consts.tile([128, 4 * TQ], BF16, tag="masks")

        qT = [
            persist.tile([128, T], BF16, tag=f"qT{p}", name=f"qT{p}") for p in range(2)
        ]
        kT = [
            persist.tile([128, T], BF16, tag=f"kT{p}", name=f"kT{p}") for p in range(2)
        ]
        yT = [
            persist.tile([128, T], BF16, tag=f"yT{p}", name=f"yT{p}") for p in range(2)
        ]
        # padded AV lhsT tiles per (key tile, pair, half); see module docstring
        vaug = persist.tile([128, NT, 2, 2, 128], BF16, tag="vaug")

        def load_consts_tail():
            nc.sync.dma_start(out=mk_sb[:], in_=masks[:])
            nc.sync.dma_start(
                out=wp_sb[:], in_=wproj[:].rearrange("a p c -> p a c")
            )
            nc.vector.memset(vaug[:], 0.0)
            for pair in range(2):
                # mk_sb[:, TQ-1] is all-ones (d=0 staircase, last column)
                ones_col = mk_sb[:, TQ - 1 : TQ].unsqueeze(1).broadcast_to(
                    (128, NT, 1)
                )
                nc.vector.tensor_copy(vaug[:, :, pair, 0, D : D + 1], ones_col)
                nc.vector.tensor_copy(vaug[:, :, pair, 1, D - 1 : D], ones_col)

        def qkv_block(tb, rep):
            x_sb = xpool.tile([128, CK, TQ], BF16, tag="x")
            first = tb == 0 and rep == 0
            if first:
                for k in range(CK):
                    nc.sync.dma_start(out=x_sb[:, k, :], in_=xt[tb, k, :, :])
                    # q/k weight columns first - they gate the first matmuls
                    nc.sync.dma_start(
                        out=w_sb[:, k, 0 : 2 * GC], in_=wqkv[k, :, 0 : 2 * GC]
                    )
                nc.sync.dma_start(
                    out=w_sb[:, :, 2 * GC : 3 * GC],
                    in_=wqkv[:, :, 2 * GC : 3 * GC].rearrange("k p c -> p k c"),
                )
                load_consts_tail()
            else:
                nc.sync.dma_start(
                    out=x_sb[:], in_=xt[tb].rearrange("k p t -> p k t")
                )
            # q^T / k^T for both head pairs. In block 0 the proj bank PJ is
            # provably idle, so ping-pong groups across PP/PJ to hide the
            # psum->sbuf copy latency on the startup critical path.
            for pair in range(2):
                for which, dest in ((0, qT), (1, kT)):
                    pq = (PP + PJ)[cnt["pp"] % 2] if first else PP[0]
                    cnt["pp"] += 1
                    for k in range(CK):
                        cols = which * GC + pair * 128
                        nc.tensor.matmul(
                            pq[:],
                            w_sb[:, k, cols : cols + 128],
                            x_sb[:, k, :],
                            start=(k == 0),
                            stop=(k == CK - 1),
                        )
                    nc.vector.tensor_copy(
                        dest[pair][:, tb * TQ : (tb + 1) * TQ], pq[:]
                    )
            # v for the 4 key tiles of this block
            for tt in range(TQ // TK):
                jt = tb * (TQ // TK) + tt
                pv = (PP + PJ)[cnt["pp"] % 2] if first else PP[0]
                cnt["pp"] += 1
                for k in range(CK):
                    nc.tensor.matmul(
                        pv[:, 0:GC],
                        x_sb[:, k, tt * TK : (tt + 1) * TK],
                        w_sb[:, k, 2 * GC : 3 * GC],
                        start=(k == 0),
                        stop=(k == CK - 1),
                    )
                for pair in range(2):
                    nc.vector.tensor_copy(
                        vaug[:, jt, pair, 0, 0:D],
                        pv[:, pair * 128 : pair * 128 + D],
                    )
                    nc.vector.tensor_copy(
                        vaug[:, jt, pair, 1, D:128],
                        pv[:, pair * 128 + D : (pair + 1) * 128],
                    )

        def attn_block(pair, i):
            jmax = (TQ // TK) * (i + 1)
            for j in range(jmax):
                dg = j - (TQ // TK) * i  # >=0 on the diagonal band
                # valid region of this tile is cols [dg*TK, TQ)
                c0 = dg * TK if dg > 0 else 0
                cs = slice(c0, TQ)
                psd = PSD[cnt["ps"] % 2]
                cnt["ps"] += 1
                for half in range(2):
                    lo, hi = half * D, half * D + D
                    kap = kT[pair][lo:hi, j * TK : (j + 1) * TK]
                    qap = qT[pair][lo:hi, i * TQ + c0 : (i + 1) * TQ]
                    nc.tensor.matmul(
                        psd[:, half, cs], kap, qap, start=True, stop=True
                    )
                est = espool.tile([128, 2, TQ], BF16, tag="es", name="est")
                nc.scalar.activation(est[:, :, cs], psd[:, :, cs], EXPF, scale=0.125)
                if dg >= 0:
                    # only the staircase strip [c0, c0+TK) needs masking:
                    # cols >= c0+TK are fully valid (p + 128*dg <= c)
                    ms = slice(c0, c0 + TK)
                    nc.vector.tensor_mul(
                        est[:, :, ms],
                        est[:, :, ms],
                        mk_sb[:, dg * TQ + c0 : dg * TQ + c0 + TK]
                        .unsqueeze(1)
                        .broadcast_to((128, 2, TK)),
                    )
                for half in range(2):
                    vap = vaug[:, j, pair, half, :]
                    nc.tensor.matmul(
                        AVD[:, half, cs],
                        vap,
                        est[:, half, cs],
                        start=(j == 0),
                        stop=(j == jmax - 1),
                    )
            # normalize: y = y_unnorm / denom.  denomA sits in psum row 64
            # (half 0), denomB in row 63 (half 1); y_A rows 0-63, y_B 64-127.
            tqs = slice(i * TQ, (i + 1) * TQ)
            avs = rpool.tile([128, 2, TQ], BF16, tag="avs", name="avs")
            nc.vector.tensor_copy(avs[:], AVD[:])
            r32 = rpool.tile([128, 2, TQ], BF16, tag="r32", name="r32")
            with nc.allow_low_precision(reason="bf16 softmax denominators"):
                nc.vector.reciprocal(r32[D - 1 : D + 1, :, :], avs[D - 1 : D + 1, :, :])
            # DRAM-bounce broadcast of 1/denom to all partitions (keeps AVD
            # free so the next attention block's AV can start immediately)
            scr = dpool.tile([2, TQ], BF16, tag="scr", name="scr")
            nc.sync.dma_start(out=scr[0:1, :], in_=r32[D : D + 1, 0, :])
            nc.sync.dma_start(out=scr[1:2, :], in_=r32[D - 1 : D, 1, :])
            rb = rbpool.tile([128, TQ], BF16, tag="rb", name="rb")
            nc.sync.dma_start(
                out=rb[:].rearrange("(a p) t -> a p t", a=2),
                in_=scr[:].unsqueeze(1).to_broadcast((2, D, TQ)),
            )
            nc.vector.tensor_mul(yT[pair][0:D, tqs], avs[0:D, 0, :], rb[0:D, :])
            nc.vector.tensor_mul(
                yT[pair][D:128, tqs], avs[D:128, 1, :], rb[D:128, :]
            )

        def proj_block(i):
            # on the last block there is no next qkv, so the PP bank is free:
            # ping-pong proj groups across PJ/PP to hide copy-waits in the tail
            last_blk = i == NB - 1
            osb = opool.tile([128, TQ // TK, C], BF16, tag="o")
            for tt in range(TQ // TK):
                tq0 = i * TQ + tt * TK
                for half in range(2):
                    po = (PJ + PP)[cnt["pj"] % 2] if last_blk else PJ[0]
                    cnt["pj"] += 1
                    for pair in range(2):
                        yap = yT[pair][:, tq0 : tq0 + TK]
                        wap = wp_sb[:, pair, half * 512 : (half + 1) * 512]
                        nc.tensor.matmul(
                            po[:],
                            yap,
                            wap,
                            start=(pair == 0),
                            stop=(pair == 1),
                        )
                    dst = osb[:, tt, half * 512 : (half + 1) * 512]
                    # balance psum evacuation between DVE and ACT
                    if (tt * 2 + half) % 2 == 0:
                        nc.vector.tensor_copy(dst, po[:])
                    else:
                        nc.scalar.copy(dst, po[:])
            nc.sync.dma_start(
                out=out[i].rearrange("(a p) c -> p a c", p=TK), in_=osb[:]
            )

        for _rep in range(repeat):
            for tb in range(NB):
                qkv_block(tb, _rep)
                for pair in range(2):
                    attn_block(pair, tb)
                proj_block(tb)

    nc.compile()
    return nc


def _host_shards(x, w_qkv, w_proj):
    import ml_dtypes

    bf16 = ml_dtypes.bfloat16
    x = np.asarray(x, dtype=np.float32)
    w_qkv = np.asarray(w_qkv, dtype=np.float32)
    w_proj = np.asarray(w_proj, dtype=np.float32)

    p = np.arange(128)[:, None]
    c = np.arange(TQ)[None, :]
    masks = np.concatenate(
        [(p + d * TK <= c).astype(np.float32) for d in range(4)], axis=1
    ).astype(bf16)  # [128, 2048]

    in_maps = []
    for core in range(NCORES):
        b, g = divmod(core, NCORES // B)
        qc = w_qkv[:, g * GC : (g + 1) * GC]
        kc = w_qkv[:, C + g * GC : C + (g + 1) * GC]
        vc = w_qkv[:, 2 * C + g * GC : 2 * C + (g + 1) * GC]
        xt = x[b].T.reshape(CK, 128, NB, TQ).transpose(2, 0, 1, 3)
        wg = np.concatenate([qc, kc, vc], axis=1).reshape(CK, 128, 3 * GC)
        wp = w_proj[g * GC : (g + 1) * GC, :].reshape(2, 128, C)
        in_maps.append(
            {
                "xt": np.ascontiguousarray(xt).astype(bf16),
                "wqkv": np.ascontiguousarray(wg).astype(bf16),
                "wproj": np.ascontiguousarray(wp).astype(bf16),
                "masks": masks,
            }
        )
    return in_maps


def kernel(x, w_qkv, w_proj, _trace=False, _trace_kwargs=None):
    if "nc" not in _CACHE:
        _CACHE["nc"] = _build_bass()
    nc = _CACHE["nc"]
    in_maps = _host_shards(x, w_qkv, w_proj)
    res = run_bass_kernel_spmd(
        nc,
        in_maps,
        core_ids=list(range(NCORES)),
        trace=_trace,
        **(_trace_kwargs or {}),
    )
    _CACHE["last_result"] = res
    g_per_b = NCORES // B
    out = np.stack(
        [
            np.sum(
                [
                    np.asarray(res.results[b * g_per_b + g]["out"], dtype=np.float32)
                    for g in range(g_per_b)
                ],
                axis=0,
            )
            for b in range(B)
        ]
    ).reshape(B, T, C).astype(np.float32)
    return out


# revision 4
# speedup vs baseline: 1.2248x; 1.2248x over previous
"""Causal self-attention (B=2, T=2048, C=1024, H=16) on 8 trn2 NeuronCores.

Sharding: core = (batch b, head-group g) with 4 heads per group.
  - data parallel over B (2 ways) x tensor parallel over heads (4 ways)
  - each core computes qkv for its head group, causal attention for its
    4 heads, and a partial proj (its 256 rows of w_proj); the host sums
    the 4 per-batch partials (deferred tensor-parallel all-reduce).

All SBUF operands are bf16 (PE streams 1 moving-elem/cycle at any size,
DVE gets 2x tensor_tensor, DMA bytes halve); PSUM accumulation is fp32.
Attention math is arranged so no on-device transposes are needed:
  q^T,k^T [d, t] come straight out of the qkv matmul (lhsT = w slice,
  rhs = x^T); S^T[tk,tq] = k^T.T @ q^T-moving; exp on ACT (one merged
  instruction per key tile covering both heads of a pair); y^T and the
  softmax denominators come from the AV matmul via padded 128-wide
  v-lhsT tiles:
    half0 lhsT = [v_h0 | ones | 0...]   -> y_A in psum rows 0-63, denomA row 64
    half1 lhsT = [0... | ones | v_h1]   -> denomB row 63, y_B rows 64-127
  so after the psum->sbuf copy both heads' y sit partition-aligned with
  yT and no cross-partition DMA is needed for the normalize.
Softmax skips max-subtraction (scores ~ N(0,1) after 1/sqrt(D): exp is
safe in fp32), matching the reference up to fp rounding.
"""

import os
import sys
from contextlib import ExitStack

import numpy as np

for _p in ("/opt/trn_rl_repo", "/root/.axon_site/_ro/trn_rl_repo"):
    if os.path.isdir(_p) and _p not in sys.path:
        sys.path.insert(0, _p)

import concourse.bass as bass
import concourse.bacc as bacc
import concourse.mybir as mybir
import concourse.tile as tile
from concourse.bass_utils import run_bass_kernel_spmd

B, T, C, H, D = 2, 2048, 1024, 16, 64
GH = 4                 # heads per core (group)
GC = GH * D            # 256 channels per group
NCORES = 8
TQ = 512               # query tile (free dim of S^T / AV matmuls)
TK = 128               # key tile (partition dim of S^T)
NB = T // TQ           # 4 query blocks
NT = T // TK           # 16 key tiles
CK = C // 128          # 8 contraction chunks for qkv
F32 = mybir.dt.float32
BF16 = mybir.dt.bfloat16

EXPF = mybir.ActivationFunctionType.Exp

_CACHE = {}


def _build_bass(repeat=1):
    nc = bacc.Bacc("TRN2", target_bir_lowering=False, debug=False)
    xt = nc.declare_dram_parameter("xt", [NB, CK, 128, TQ], BF16, isOutput=False)
    wqkv = nc.declare_dram_parameter("wqkv", [CK, 128, 3 * GC], BF16, isOutput=False)
    wproj = nc.declare_dram_parameter("wproj", [2, 128, C], BF16, isOutput=False)
    masks = nc.declare_dram_parameter("masks", [128, 4 * TQ], BF16, isOutput=False)
    out = nc.declare_dram_parameter("out", [NB, TQ, C], BF16, isOutput=True)

    with ExitStack() as ctx:
        tc = ctx.enter_context(tile.TileContext(nc))
        consts = ctx.enter_context(tc.tile_pool(name="consts", bufs=1))
        persist = ctx.enter_context(tc.tile_pool(name="persist", bufs=1))
        xpool = ctx.enter_context(tc.tile_pool(name="xp", bufs=2))
        espool = ctx.enter_context(tc.tile_pool(name="es", bufs=4))
        rpool = ctx.enter_context(tc.tile_pool(name="rp", bufs=2))
        rbpool = ctx.enter_context(tc.tile_pool(name="rb", bufs=2))
        opool = ctx.enter_context(tc.tile_pool(name="op", bufs=2))
        dpool = ctx.enter_context(tc.tile_pool(name="dp", bufs=2, space="DRAM"))
        # 8 fixed PSUM banks, rotated manually (pool-based PSUM slot reuse
        # emits extra release waits on the claiming matmul).
        psum = ctx.enter_context(tc.tile_pool(name="psum", bufs=1, space="PSUM"))
        PSD = [
            psum.tile([128, 2, TQ], F32, tag=f"psd{r}", name=f"psd{r}")
            for r in range(2)
        ]
        PP = [psum.tile([128, TQ], F32, tag="pp0", name="pp0")]
        PJ = [psum.tile([128, TQ], F32, tag="pj0", name="pj0")]
        AVD = psum.tile([128, 2, TQ], F32, tag="avd", name="avd")
        cnt = {"pp": 0, "pj": 0, "ps": 0}

        # ---- constants / persistent tiles ----
        w_sb = consts.tile([128, CK, 3 * GC], BF16, tag="wqkv")
        wp_sb = consts.tile([128, 2, C], BF16, tag="wproj")
        mk_sb = consts.tile([128, 4 * TQ], BF16, tag="masks")

        qT = [
            persist.tile([128, T], BF16, tag=f"qT{p}", name=f"qT{p}") for p in range(2)
        ]
        kT = [
            persist.tile([128, T], BF16, tag=f"kT{p}", name=f"kT{p}") for p in range(2)
        ]
        yT = [
            persist.tile([128, T], BF16, tag=f"yT{p}", name=f"yT{p}") for p in range(2)
        ]
        # padded AV lhsT tiles per (key tile, pair, half); see module docstring
        vaug = persist.tile([128, NT, 2, 2, 128], BF16, tag="vaug")

        def load_consts_tail():
            nc.sync.dma_start(out=mk_sb[:], in_=masks[:])
            nc.sync.dma_start(
                out=wp_sb[:], in_=wproj[:].rearrange("a p c -> p a c")
            )
            nc.vector.memset(vaug[:], 0.0)
            for pair in range(2):
                # mk_sb[:, TQ-1] is all-ones (d=0 staircase, last column)
                ones_col = mk_sb[:, TQ - 1 : TQ].unsqueeze(1).broadcast_to(
                    (128, NT, 1)
                )
                nc.vector.tensor_copy(vaug[:, :, pair, 0, D : D + 1], ones_col)
                nc.vector.tensor_copy(vaug[:, :, pair, 1, D - 1 : D], ones_col)

        def qkv_block(tb, rep):
            x_sb = xpool.tile([128, CK, TQ], BF16, tag="x")
            first = tb == 0 and rep == 0
            if first:
                for k in range(CK):
                    nc.sync.dma_start(out=x_sb[:, k, :], in_=xt[tb, k, :, :])
                    # q/k weight columns first - they gate the first matmuls
                    nc.sync.dma_start(
                        out=w_sb[:, k, 0 : 2 * GC], in_=wqkv[k, :, 0 : 2 * GC]
                    )
                nc.sync.dma_start(
                    out=w_sb[:, :, 2 * GC : 3 * GC],
                    in_=wqkv[:, :, 2 * GC : 3 * GC].rearrange("k p c -> p k c"),
                )
                load_consts_tail()
            else:
                # two half-loads bound head-of-line blocking on the DMA queue
                for h in range(2):
                    ks = slice(h * (CK // 2), (h + 1) * (CK // 2))
                    nc.sync.dma_start(
                        out=x_sb[:, ks, :],
                        in_=xt[tb, ks].rearrange("k p t -> p k t"),
                    )
            # q^T / k^T for both head pairs. In block 0 the proj bank PJ is
            # provably idle, so ping-pong groups across PP/PJ to hide the
            # psum->sbuf copy latency on the startup critical path.
            for pair in range(2):
                for which, dest in ((0, qT), (1, kT)):
                    pq = (PP + PJ)[cnt["pp"] % 2] if first else PP[0]
                    cnt["pp"] += 1
                    for k in range(CK):
                        cols = which * GC + pair * 128
                        nc.tensor.matmul(
                            pq[:],
                            w_sb[:, k, cols : cols + 128],
                            x_sb[:, k, :],
                            start=(k == 0),
                            stop=(k == CK - 1),
                        )
                    nc.vector.tensor_copy(
                        dest[pair][:, tb * TQ : (tb + 1) * TQ], pq[:]
                    )
            # v for the 4 key tiles of this block
            for tt in range(TQ // TK):
                jt = tb * (TQ // TK) + tt
                pv = (PP + PJ)[cnt["pp"] % 2] if first else PP[0]
                cnt["pp"] += 1
                for k in range(CK):
                    nc.tensor.matmul(
                        pv[:, 0:GC],
                        x_sb[:, k, tt * TK : (tt + 1) * TK],
                        w_sb[:, k, 2 * GC : 3 * GC],
                        start=(k == 0),
                        stop=(k == CK - 1),
                    )
                for pair in range(2):
                    nc.vector.tensor_copy(
                        vaug[:, jt, pair, 0, 0:D],
                        pv[:, pair * 128 : pair * 128 + D],
                    )
                    nc.vector.tensor_copy(
                        vaug[:, jt, pair, 1, D:128],
                        pv[:, pair * 128 + D : (pair + 1) * 128],
                    )

        def attn_block(pair, i):
            jmax = (TQ // TK) * (i + 1)
            for j in range(jmax):
                dg = j - (TQ // TK) * i  # >=0 on the diagonal band
                # valid region of this tile is cols [dg*TK, TQ)
                c0 = dg * TK if dg > 0 else 0
                cs = slice(c0, TQ)
                psd = PSD[cnt["ps"] % 2]
                cnt["ps"] += 1
                for half in range(2):
                    lo, hi = half * D, half * D + D
                    kap = kT[pair][lo:hi, j * TK : (j + 1) * TK]
                    qap = qT[pair][lo:hi, i * TQ + c0 : (i + 1) * TQ]
                    nc.tensor.matmul(
                        psd[:, half, cs], kap, qap, start=True, stop=True
                    )
                est = espool.tile([128, 2, TQ], BF16, tag="es", name="est")
                nc.scalar.activation(est[:, :, cs], psd[:, :, cs], EXPF, scale=0.125)
                if dg >= 0:
                    # only the staircase strip [c0, c0+TK) needs masking:
                    # cols >= c0+TK are fully valid (p + 128*dg <= c)
                    ms = slice(c0, c0 + TK)
                    nc.vector.tensor_mul(
                        est[:, :, ms],
                        est[:, :, ms],
                        mk_sb[:, dg * TQ + c0 : dg * TQ + c0 + TK]
                        .unsqueeze(1)
                        .broadcast_to((128, 2, TK)),
                    )
                for half in range(2):
                    vap = vaug[:, j, pair, half, :]
                    nc.tensor.matmul(
                        AVD[:, half, cs],
                        vap,
                        est[:, half, cs],
                        start=(j == 0),
                        stop=(j == jmax - 1),
                    )
            # normalize: y = y_unnorm / denom.  denomA sits in psum row 64
            # (half 0), denomB in row 63 (half 1); y_A rows 0-63, y_B 64-127.
            tqs = slice(i * TQ, (i + 1) * TQ)
            avs = rpool.tile([128, 2, TQ], BF16, tag="avs", name="avs")
            nc.vector.tensor_copy(avs[:], AVD[:])
            r32 = rpool.tile([128, 2, TQ], BF16, tag="r32", name="r32")
            with nc.allow_low_precision(reason="bf16 softmax denominators"):
                nc.vector.reciprocal(r32[D - 1 : D + 1, :, :], avs[D - 1 : D + 1, :, :])
            # broadcast 1/denom to all partitions on the (otherwise idle)
            # GPSIMD engine; keeps AVD free and avoids DMA round-trips
            rb = rbpool.tile([128, TQ], BF16, tag="rb", name="rb")
            nc.gpsimd.partition_broadcast(rb[0:D, :], r32[D : D + 1, 0, :])
            nc.gpsimd.partition_broadcast(rb[D:128, :], r32[D - 1 : D, 1, :])
            nc.vector.tensor_mul(yT[pair][0:D, tqs], avs[0:D, 0, :], rb[0:D, :])
            nc.vector.tensor_mul(
                yT[pair][D:128, tqs], avs[D:128, 1, :], rb[D:128, :]
            )

        def proj_block(i):
            # on the last block there is no next qkv, so the PP bank is free:
            # ping-pong proj groups across PJ/PP to hide copy-waits in the tail
            last_blk = i == NB - 1
            osb = opool.tile([128, TQ // TK, C], BF16, tag="o")
            for tt in range(TQ // TK):
                tq0 = i * TQ + tt * TK
                for half in range(2):
                    po = (PJ + PP)[cnt["pj"] % 2] if last_blk else PJ[0]
                    cnt["pj"] += 1
                    for pair in range(2):
                        yap = yT[pair][:, tq0 : tq0 + TK]
                        wap = wp_sb[:, pair, half * 512 : (half + 1) * 512]
                        nc.tensor.matmul(
                            po[:],
                            yap,
                            wap,
                            start=(pair == 0),
                            stop=(pair == 1),
                        )
                    dst = osb[:, tt, half * 512 : (half + 1) * 512]
                    # balance psum evacuation between DVE and ACT
                    if (tt * 2 + half) % 2 == 0:
                        nc.vector.tensor_copy(dst, po[:])
                    else:
                        nc.scalar.copy(dst, po[:])
            nc.sync.dma_start(
                out=out[i].rearrange("(a p) c -> p a c", p=TK), in_=osb[:]
            )

        for _rep in range(repeat):
            for tb in range(NB):
                qkv_block(tb, _rep)
                for pair in range(2):
                    attn_block(pair, tb)
                proj_block(tb)

    nc.compile()
    return nc


def _host_shards(x, w_qkv, w_proj):
    import ml_dtypes

    bf16 = ml_dtypes.bfloat16
    x = np.asarray(x, dtype=np.float32)
    w_qkv = np.asarray(w_qkv, dtype=np.float32)
    w_proj = np.asarray(w_proj, dtype=np.float32)

    p = np.arange(128)[:, None]
    c = np.arange(TQ)[None, :]
    masks = np.concatenate(
        [(p + d * TK <= c).astype(np.float32) for d in range(4)], axis=1
    ).astype(bf16)  # [128, 2048]

    in_maps = []
    for core in range(NCORES):
        b, g = divmod(core, NCORES // B)
        qc = w_qkv[:, g * GC : (g + 1) * GC]
        kc = w_qkv[:, C + g * GC : C + (g + 1) * GC]
        vc = w_qkv[:, 2 * C + g * GC : 2 * C + (g + 1) * GC]
        xt = x[b].T.reshape(CK, 128, NB, TQ).transpose(2, 0, 1, 3)
        wg = np.concatenate([qc, kc, vc], axis=1).reshape(CK, 128, 3 * GC)
        wp = w_proj[g * GC : (g + 1) * GC, :].reshape(2, 128, C)
        in_maps.append(
            {
                "xt": np.ascontiguousarray(xt).astype(bf16),
                "wqkv": np.ascontiguousarray(wg).astype(bf16),
                "wproj": np.ascontiguousarray(wp).astype(bf16),
                "masks": masks,
            }
        )
    return in_maps


def kernel(x, w_qkv, w_proj, _trace=False, _trace_kwargs=None):
    if "nc" not in _CACHE:
        _CACHE["nc"] = _build_bass()
    nc = _CACHE["nc"]
    in_maps = _host_shards(x, w_qkv, w_proj)
    res = run_bass_kernel_spmd(
        nc,
        in_maps,
        core_ids=list(range(NCORES)),
        trace=_trace,
        **(_trace_kwargs or {}),
    )
    _CACHE["last_result"] = res
    g_per_b = NCORES // B
    out = np.stack(
        [
            np.sum(
                [
                    np.asarray(res.results[b * g_per_b + g]["out"], dtype=np.float32)
                    for g in range(g_per_b)
                ],
                axis=0,
            )
            for b in range(B)
        ]
    ).reshape(B, T, C).astype(np.float32)
    return out
